# revision 19
# baseline (speedup 1.0000x reference)
"""Trainium2 Bass kernel: chunked sliding-window attention block (8-core SPMD).

Model (reference): q/k/v/o projections (1024->1024, 16 heads x 64) + causal
sliding-window attention, window=128, over x [2, 4096, 1024] fp32.

Sharding: 8 shards over (batch, seq): core c -> batch c//4, positions
[(c%4)*1024, +1024). Each core also receives a 128-position halo of x (the
previous window); the first shard of each batch gets a zeroed halo plus a
zeroed "flag" column so halo keys contribute nothing to the softmax sums.

All-bf16 operands with fp32 PSUM accumulation; output stored bf16 and
widened on host (rel err ~4e-3 vs the 2e-2 gate).

Per-core pipeline (built with Tile, persistent pools across reps):
  phase 1: K^T = Wk @ X^T, Q^T = (Wq/8) @ X^T  ([dims, pos] layouts),
           V = X @ Wv^T ([pos, dims] + flag/ones column for softmax sums)
  phase 2: per chunk of 128 queries, per head pair: S^T = K-chunk^T.T @ Q^T
           ([keys, queries]), P^T = exp(S^T) (ScalarE, bf16), band mask
           applied post-exp via affine_select (binary zeroing, DVE/GpSimd),
           O = P^T.T @ [V|1] (row-sums ride along), A = O * 1/rowsum (DVE),
           A^T via PE transpose, out = A^T.T @ Wo^T, bf16 store.

Scheduling: input loads ride the Activation HWDGE queue (one descriptor per
tensor), stores the SP queue; SBUF/PSUM pools persist across reps so the
next repetition's loads prefetch during this one's attention phase; the
timing loop unrolls 4 reps per For_i iteration to amortize the loop's
all-engine barrier.
"""
import numpy as np
import ml_dtypes

import concourse.bass as bass
import concourse.tile as tile
from concourse import bacc, mybir
from concourse.bass_utils import run_bass_kernel_spmd
from concourse.masks import make_identity

B = 2
S = 4096
D = 1024
H = 16
DH = 64
W = 128
N_CORES = 8
SHARDS_PER_B = N_CORES // B
OWN = S // SHARDS_PER_B          # 1024 positions per core
HALO = W                          # 128
LOC = OWN + HALO                  # 1152 positions incl. halo
NCHUNK = OWN // W                 # 8 query chunks per core

F32 = mybir.dt.float32
BF16 = mybir.dt.bfloat16
GE = mybir.AluOpType.is_ge
EXP = mybir.ActivationFunctionType.Exp


def build_nc(reps: int = 1):
    nc = bacc.Bacc()
    xt_d = nc.declare_dram_parameter("xt", [D, LOC], BF16, isOutput=False)
    wqt_d = nc.declare_dram_parameter("wqt", [D, D], BF16, isOutput=False)
    wkt_d = nc.declare_dram_parameter("wkt", [D, D], BF16, isOutput=False)
    wvt_d = nc.declare_dram_parameter("wvt", [D, D], BF16, isOutput=False)
    wot_d = nc.declare_dram_parameter("wot", [D, D], BF16, isOutput=False)
    flag_d = nc.declare_dram_parameter("flag", [128, H], BF16, isOutput=False)
    out_d = nc.declare_dram_parameter("out", [OWN, D], BF16, isOutput=True)

    UNROLL = 4
    with tile.TileContext(nc) as tc:
        import contextlib
        with contextlib.ExitStack() as stk:
            pools = {}
            def pool(name, bufs, space="SBUF"):
                pools[name] = stk.enter_context(
                    tc.tile_pool(name=name, bufs=bufs, space=space))
            pool("const", 1)
            pool("w", 1)        # wk/wq/wv/wo resident slots (reloaded per rep)
            pool("xt", 1)       # persistent: next rep's load starts mid-rep
            pool("kt", 1)
            pool("v", 1)
            pool("ptb", 3)
            pool("a", 2)
            pool("at", 2)
            pool("rd", 4)
            pool("ob", 2)
            pool("proj_ps", 2, space="PSUM")   # [128,512] f32: 1 bank x2
            pool("sc_ps", 2, space="PSUM")     # [128,2,256] f32: 1 bank x2
            pool("pv_ps", 2, space="PSUM")     # [128,4,128] f32: 1 bank x2
            pool("tr_ps", 1, space="PSUM")     # [128,4,128] bf16: 1 bank
            pool("out_ps", 1, space="PSUM")    # [128,512] f32: 1 bank

            ident_bf = pools["const"].tile([128, 128], BF16, tag="id")
            make_identity(nc, ident_bf)
            # binary band mask [keys, 2 heads, 2W queries], built once:
            # cols 0:W   (key chunk t vs query chunk t-1): causal, valid q >= k
            # cols W:2W  (key chunk t vs query chunk t):   strict, valid k >= q+1
            band_mask = pools["const"].tile([128, 2, 2 * W], BF16, tag="bm")
            nc.vector.memset(band_mask, 1.0)
            nc.gpsimd.affine_select(
                out=band_mask[:, :, 0:W], in_=band_mask[:, :, 0:W],
                compare_op=GE, fill=0.0,
                base=0, channel_multiplier=-1, pattern=[[0, 2], [1, W]])
            nc.gpsimd.affine_select(
                out=band_mask[:, :, W:2 * W], in_=band_mask[:, :, W:2 * W],
                compare_op=GE, fill=0.0,
                base=-1, channel_multiplier=1, pattern=[[0, 2], [-1, W]])
            # zero-padded Q^T: slot h holds head h's 64 dims in its h%2
            # partition half, zeros in the other half. Lets score matmuls
            # contract a full 128 partitions from partition 0 (start=True at
            # a 64-row tile position faults the runtime). Zeros written once.
            qt_z = pools["const"].tile([128, H, OWN], BF16, tag="qtz")
            for h in range(H):
                if h % 2 == 0:
                    nc.vector.memset(qt_z[64:128, h, :], 0.0)
                else:
                    nc.vector.memset(qt_z[0:64, h, :], 0.0)

            args = (nc, tc, pools, xt_d, wqt_d, wkt_d, wvt_d, wot_d, flag_d,
                    out_d, ident_bf, band_mask, qt_z)
            if reps <= UNROLL:
                for _ in range(reps):
                    _build_rep(*args)
            else:
                engines = [mybir.EngineType.PE, mybir.EngineType.Activation,
                           mybir.EngineType.DVE, mybir.EngineType.SP,
                           mybir.EngineType.Pool]
                with tc.For_i(0, reps // UNROLL, 1, hint_engines=tuple(engines)):
                    for _ in range(UNROLL):
                        _build_rep(*args)
                for _ in range(reps % UNROLL):
                    _build_rep(*args)
    nc.compile()
    return nc


def _build_rep(nc, tc, pools, xt_d, wqt_d, wkt_d, wvt_d, wot_d, flag_d,
               out_d, ident_bf, band_mask, qt_z):
    wpool, xtp = pools["w"], pools["xt"]
    kt_sb = pools["kt"].tile([128, 8, LOC], BF16, tag="kt")
    v_sb = pools["v"].tile([128, LOC // W, H, 68], BF16, tag="v")
    proj_ps = pools["proj_ps"]

    # ---------------- input loads (Act HWDGE queue) ----------------
    wk_sb = wpool.tile([128, 8, D], BF16, tag="wk")
    xt_sb = xtp.tile([128, 8, LOC], BF16, tag="xt")
    for k in range(8):
        nc.scalar.dma_start(out=wk_sb[:, k, :], in_=wkt_d[k * 128:(k + 1) * 128, :])
        nc.scalar.dma_start(out=xt_sb[:, k, :], in_=xt_d[k * 128:(k + 1) * 128, :])
    wq_sb = wpool.tile([128, 8, D], BF16, tag="wq")
    for k in range(8):
        nc.scalar.dma_start(out=wq_sb[:, k, :], in_=wqt_d[k * 128:(k + 1) * 128, :])
    wv_sb = wpool.tile([128, 8, D], BF16, tag="wv")
    for k in range(8):
        nc.scalar.dma_start(out=wv_sb[:, k, :], in_=wvt_d[k * 128:(k + 1) * 128, :])
    # softmax-denominator column: 1.0 for own chunks, per-core flag for halo
    nc.vector.memset(v_sb[:, 1:, :, 64:65], 1.0)
    flag_sb = pools["const"].tile([128, H], BF16, tag="flag")
    nc.scalar.dma_start(out=flag_sb, in_=flag_d[:, :])
    nc.vector.tensor_copy(v_sb[:, 0, :, 64:65], flag_sb.rearrange("p (h o) -> p h o", o=1))
    wo_sb = wpool.tile([128, 8, D], BF16, tag="wo")
    for k in range(8):
        nc.scalar.dma_start(out=wo_sb[:, k, :], in_=wot_d[k * 128:(k + 1) * 128, :])

    ncopy = 0

    def copy_out(dst, src):
        nonlocal ncopy
        if ncopy % 2 == 0:
            nc.vector.tensor_copy(dst, src)
        else:
            nc.scalar.copy(dst, src)
        ncopy += 1

    # ---------------- phase 1: projections ----------------
    # K^T [dims, all LOC positions]
    for m in range(8):
        for off, width in ((0, 512), (512, 512), (1024, 128)):
            p = proj_ps.tile([128, 512], F32, tag="pp")
            for k in range(8):
                nc.tensor.matmul(
                    p[:, :width],
                    wk_sb[:, k, m * 128:(m + 1) * 128],
                    xt_sb[:, k, off:off + width],
                    start=(k == 0), stop=(k == 7),
                )
            copy_out(kt_sb[:, m, off:off + width], p[:, :width])

    # Q^T [dims, own positions] (1/8 scale folded into Wq host-side),
    # split per head into qt_z's zero-padded slots
    for m in range(8):
        for n in range(2):
            p = proj_ps.tile([128, 512], F32, tag="pp")
            for k in range(8):
                nc.tensor.matmul(
                    p,
                    wq_sb[:, k, m * 128:(m + 1) * 128],
                    xt_sb[:, k, HALO + n * 512:HALO + (n + 1) * 512],
                    start=(k == 0), stop=(k == 7),
                )
            ncols = slice(n * 512, (n + 1) * 512)
            copy_out(qt_z[0:64, 2 * m, ncols], p[0:64, :])
            copy_out(qt_z[64:128, 2 * m + 1, ncols], p[64:128, :])

    # V natural [pos, dims] (bf16)
    for t in range(LOC // W):
        for n in range(2):
            p = proj_ps.tile([128, 512], F32, tag="pp")
            for k in range(8):
                nc.tensor.matmul(
                    p,
                    xt_sb[:, k, t * 128:(t + 1) * 128],
                    wv_sb[:, k, n * 512:(n + 1) * 512],
                    start=(k == 0), stop=(k == 7),
                )
            copy_out(
                v_sb[:, t, n * 8:(n + 1) * 8, 0:64],
                p.rearrange("p (h d) -> p h d", h=8))

    # ---------------- phase 2: attention + out-projection ----------------
    sc_ps, pv_ps = pools["sc_ps"], pools["pv_ps"]
    tr_ps, out_ps = pools["tr_ps"], pools["out_ps"]
    ptb_p, apool, atp, rdp, obp = (pools["ptb"], pools["a"], pools["at"],
                                   pools["rd"], pools["ob"])
    ptbs = {}
    a_cs = {}
    for t in range(NCHUNK + 4):
        if t <= NCHUNK:
            qlo = max(t - 1, 0) * W
            qhi = min(t + 1, NCHUNK) * W
            width = qhi - qlo
            dst0 = W if t == 0 else 0
            ptb = ptb_p.tile([128, H, 2 * W], BF16, tag="ptb")
            ptbs[t] = ptb
            for pr in range(8):
                sc2 = sc_ps.tile([128, 2, 2 * W], F32, tag="sc")
                for i in range(2):
                    nc.tensor.matmul(
                        sc2[:, i, dst0:dst0 + width],
                        kt_sb[:, pr, t * W:(t + 1) * W],
                        qt_z[:, 2 * pr + i, qlo:qhi],
                        start=True, stop=True,
                    )
                nc.scalar.activation(
                    ptb[:, 2 * pr:2 * pr + 2, dst0:dst0 + width],
                    sc2[:, :, dst0:dst0 + width],
                    EXP,
                )
                # band mask: post-exp binary zeroing against the constant mask
                nc.vector.tensor_mul(
                    ptb[:, 2 * pr:2 * pr + 2, dst0:dst0 + width],
                    ptb[:, 2 * pr:2 * pr + 2, dst0:dst0 + width],
                    band_mask[:, :, dst0:dst0 + width],
                )
        if 2 <= t <= NCHUNK + 1:
            c = t - 2
            a_c = apool.tile([W, D], BF16, tag="a")
            a_cs[c] = a_c
            pp, pc = ptbs[c], ptbs.get(c + 1)
            for pr in range(8):
                if pr % 2 == 0:
                    pv4 = pv_ps.tile([W, 4, 128], F32, tag="pv")
                for i in range(2):
                    h = 2 * pr + i
                    s = (pr % 2) * 2 + i
                    nc.tensor.matmul(
                        pv4[:, s, 0:65],
                        pp[:, h, W:2 * W],
                        v_sb[:, c, h, 0:65],
                        start=True, stop=False,
                    )
                    nc.tensor.matmul(
                        pv4[:, s, 0:65],
                        pc[:, h, 0:W],
                        v_sb[:, c + 1, h, 0:65],
                        start=False, stop=True,
                    )
                if pr % 2 == 1:
                    rd4 = rdp.tile([W, 4, 1], F32, tag="rd")
                    nc.vector.reciprocal(rd4, pv4[:, :, 64:65])
                    for s in range(4):
                        h = 2 * pr - 2 + s
                        nc.vector.tensor_scalar_mul(
                            a_c[:, h * DH:(h + 1) * DH], pv4[:, s, 0:64],
                            rd4[:, s, :])
        if 3 <= t <= NCHUNK + 2:
            c = t - 3
            a_c = a_cs.pop(c)
            at = atp.tile([128, 8, W], BF16, tag="at")
            for g in range(2):
                tp = tr_ps.tile([128, 4, 128], BF16, tag="tp")
                for j in range(4):
                    nc.tensor.transpose(
                        tp[:, j, :],
                        a_c[:, (4 * g + j) * 128:(4 * g + j + 1) * 128],
                        ident_bf)
                nc.vector.tensor_copy(at[:, 4 * g:4 * g + 4, :], tp)
            ob = obp.tile([128, 2, 512], BF16, tag="ob")
            for n in range(2):
                op = out_ps.tile([128, 512], F32, tag="op")
                for j in range(8):
                    nc.tensor.matmul(
                        op, at[:, j, :], wo_sb[:, j, n * 512:(n + 1) * 512],
                        start=(j == 0), stop=(j == 7),
                    )
                nc.scalar.copy(ob[:, n, :], op)
                nc.sync.dma_start(
                    out=out_d[c * W:(c + 1) * W, n * 512:(n + 1) * 512],
                    in_=ob[:, n, :])


def shard_inputs(x, Wq, Wk, Wv, Wo):
    """Host-side prep: per-core input dicts (bf16 operands)."""
    bf = ml_dtypes.bfloat16
    wqt = np.ascontiguousarray((Wq.T * 0.125).astype(bf))
    wkt = np.ascontiguousarray(Wk.T.astype(bf))
    wvt = np.ascontiguousarray(Wv.T.astype(bf))
    wot = np.ascontiguousarray(Wo.T.astype(bf))
    in_maps = []
    for c in range(N_CORES):
        b, s0 = c // SHARDS_PER_B, (c % SHARDS_PER_B) * OWN
        xs = np.zeros((LOC, D), np.float32)
        lo = max(0, s0 - HALO)
        xs[HALO - (s0 - lo):] = x[b, lo:s0 + OWN]
        flag = np.full((128, H), 0.0 if s0 == 0 else 1.0, np.float32)
        in_maps.append({
            "xt": np.ascontiguousarray(xs.T.astype(bf)),
            "wqt": wqt, "wkt": wkt, "wvt": wvt, "wot": wot,
            "flag": flag.astype(bf),
        })
    return in_maps


_NC_CACHE = {}


def _get_nc(reps=1):
    if reps not in _NC_CACHE:
        _NC_CACHE[reps] = build_nc(reps)
    return _NC_CACHE[reps]


def kernel(x, Wq, Wk, Wv, Wo):
    x = np.asarray(x, dtype=np.float32)
    in_maps = shard_inputs(
        x, np.asarray(Wq, np.float32), np.asarray(Wk, np.float32),
        np.asarray(Wv, np.float32), np.asarray(Wo, np.float32))
    nc = _get_nc(1)
    try:
        res = run_bass_kernel_spmd(nc, in_maps, core_ids=list(range(N_CORES)))
    except Exception:
        # transient NRT device-state failures recover on retry
        res = run_bass_kernel_spmd(nc, in_maps, core_ids=list(range(N_CORES)))
    out = np.empty((B, S, D), np.float32)
    for c in range(N_CORES):
        b, s0 = c // SHARDS_PER_B, (c % SHARDS_PER_B) * OWN
        out[b, s0:s0 + OWN] = res.results[c]["out"].astype(np.float32)
    return out
